# revision 5
# baseline (speedup 1.0000x reference)
"""Grouped MoE MLP (SwiGLU) kernel for Trainium2, 8 NeuronCores.

Strategy (expert-parallel, host-side routing):
  Tokens arrive pre-sorted by expert with per-expert counts.  The host
  partitions each expert's token block into pieces matching a fixed
  per-core slot structure (preferring the zero-padding mixed cover
  (1024, 512, 256, 256) = 2048 rows/core, falling back to a uniform
  768-row scheme), and gathers the matching expert weights per
  (core, slot).  Every core runs the identical program: for each slot,
  a dense SwiGLU MLP of that slot's tokens with that slot's expert
  weights.  No device-side routing or collectives are needed.

  Layouts are transposed on the host so both GEMMs contract over the
  SBUF partition dimension with no on-chip transposes:
    GEMM1: out1^T[f, t] = sum_h W1[h, f] * x[t, h]   (h on partitions)
    SwiGLU on feature-partitioned tiles
    GEMM2: out^T[o, t]  = sum_f W2[f, o] * h[t, f]   (f on partitions)

  Weights are packed on the host into per-128-column blocks
  (w1: [slot, 22, P, KH, 128], w2: [slot, 16, P, KI, 128]) and loaded
  on-chip as small 0.72MB/0.36MB tiles in deep rings.  GEMM1 runs
  mp-outer / chunk-inner so each weight block's last use comes early
  and the ring continuously prefetches across slot boundaries --
  keeping the PE array busy (and the HAM clock-gate warm) end to end.
  Output is stored bf16 to halve the output DMA traffic.
"""

import math
from contextlib import ExitStack

import ml_dtypes
import numpy as np

P = 128
HIDDEN = 2048
INTER = 1408
GU = 2 * INTER            # 2816 = gate+up columns
KH = HIDDEN // P          # 16 k-tiles for GEMM1
KI = INTER // P           # 11 k-tiles for GEMM2 / gate-up pair blocks
MO = HIDDEN // P          # 16 output feature blocks
GB = GU // P              # 22 gate+up column blocks
N_CORES = 8
NT = 512                  # max tokens per chunk (matmul moving free dim)
# zero-padding cover, 2048 rows/core.  Smallest slots first: the small
# slots are locally DMA-bound (weight load ~51us vs ~56us compute), so
# they run while the DMA stream has the most slack ahead of it, and the
# big compute-bound slot forms a dense tail that keeps the PE warm.
MIXED_SLOTS = (256, 256, 512, 1024)
UNIFORM_SLOT = 768                    # fallback slot size

BF16 = ml_dtypes.bfloat16

_PROGRAM_CACHE: dict = {}


def _chunks(slot_rows: int, nt: int):
    out = []
    r = 0
    while r < slot_rows:
        c = min(nt, slot_rows - r)
        out.append((r, c))
        r += c
    return out


def _build_program(slot_sizes: tuple, nt: int):
    import concourse.mybir as mybir
    import concourse.tile as tile
    from concourse import bacc

    n_slots = len(slot_sizes)
    T = sum(slot_sizes)
    slot_off = np.concatenate([[0], np.cumsum(slot_sizes)]).astype(int)
    bf16 = mybir.dt.bfloat16
    f32 = mybir.dt.float32

    nc = bacc.Bacc(None, target_bir_lowering=False, debug=False)
    xT = nc.dram_tensor("xT", [P, KH, T], bf16, kind="ExternalInput")
    w1 = nc.dram_tensor("w1", [n_slots, GB, P, KH, P], bf16, kind="ExternalInput")
    w2 = nc.dram_tensor("w2", [n_slots, MO, P, KI, P], bf16, kind="ExternalInput")
    outT = nc.dram_tensor("outT", [P, MO, T], bf16, kind="ExternalOutput")

    with tile.TileContext(nc) as tc, ExitStack() as ctx:
        w1_pool = ctx.enter_context(tc.tile_pool(name="w1p", bufs=6))
        w2_pool = ctx.enter_context(tc.tile_pool(name="w2p", bufs=4))
        x_pool = ctx.enter_context(tc.tile_pool(name="xp", bufs=4))
        h_pool = ctx.enter_context(tc.tile_pool(name="hp", bufs=2))
        g_pool = ctx.enter_context(tc.tile_pool(name="gp", bufs=3))
        o_pool = ctx.enter_context(tc.tile_pool(name="op", bufs=4))
        ps1 = ctx.enter_context(tc.tile_pool(name="ps1", bufs=2, space="PSUM"))
        ps2 = ctx.enter_context(tc.tile_pool(name="ps2", bufs=4, space="PSUM"))

        for s in range(n_slots):
            sz = slot_sizes[s]
            chunk_list = _chunks(sz, nt)
            base = int(slot_off[s])

            # DMA ring split: w1 weight stream owns the Sync HWDGE FIFO;
            # x / w2 / out ride the Scalar HWDGE FIFO so they never queue
            # ahead of the weight prefetch.
            wg0 = w1_pool.tile([P, KH, P], bf16, tag="w1")
            nc.sync.dma_start(wg0[:], w1[s, 0, :, :, :])
            wu0 = w1_pool.tile([P, KH, P], bf16, tag="w1")
            nc.sync.dma_start(wu0[:], w1[s, KI, :, :, :])

            xts = []
            for c_off, c_n in chunk_list:
                xt = x_pool.tile([P, KH, c_n], bf16, tag="xt")
                nc.scalar.dma_start(xt[:], xT[:, :, base + c_off : base + c_off + c_n])
                xts.append(xt)

            ht = h_pool.tile([P, KI, sz], bf16, tag="ht")
            for mp in range(KI):
                if mp == 0:
                    wg, wu = wg0, wu0
                else:
                    wg = w1_pool.tile([P, KH, P], bf16, tag="w1")
                    nc.sync.dma_start(wg[:], w1[s, mp, :, :, :])
                    wu = w1_pool.tile([P, KH, P], bf16, tag="w1")
                    nc.sync.dma_start(wu[:], w1[s, KI + mp, :, :, :])
                for ci, (c_off, c_n) in enumerate(chunk_list):
                    xt = xts[ci]
                    pg = ps1.tile([P, c_n], f32, tag="pg")
                    pu = ps1.tile([P, c_n], f32, tag="pu")
                    for k in range(KH):
                        nc.tensor.matmul(
                            pg[:], wg[:, k, :], xt[:, k, :],
                            start=(k == 0), stop=(k == KH - 1),
                        )
                    for k in range(KH):
                        nc.tensor.matmul(
                            pu[:], wu[:, k, :], xt[:, k, :],
                            start=(k == 0), stop=(k == KH - 1),
                        )
                    gt = g_pool.tile([P, c_n], bf16, tag="gt")
                    nc.scalar.activation(
                        gt[:], pg[:], mybir.ActivationFunctionType.Silu
                    )
                    nc.vector.tensor_mul(
                        ht[:, mp, c_off : c_off + c_n], gt[:], pu[:]
                    )

            for m in range(MO):
                w2m = w2_pool.tile([P, KI, P], bf16, tag="w2")
                nc.scalar.dma_start(w2m[:], w2[s, m, :, :, :])
                for c_off, c_n in chunk_list:
                    po = ps2.tile([P, c_n], f32, tag="po")
                    for k in range(KI):
                        nc.tensor.matmul(
                            po[:], w2m[:, k, :], ht[:, k, c_off : c_off + c_n],
                            start=(k == 0), stop=(k == KI - 1),
                        )
                    om = o_pool.tile([P, c_n], bf16, tag="om")
                    nc.vector.tensor_copy(om[:], po[:])
                    nc.scalar.dma_start(
                        outT[:, m, base + c_off : base + c_off + c_n], om[:]
                    )
    nc.compile()
    return nc


def _get_program(slot_sizes: tuple, nt: int):
    key = (tuple(slot_sizes), nt)
    if key not in _PROGRAM_CACHE:
        _PROGRAM_CACHE[key] = _build_program(tuple(slot_sizes), nt)
    return _PROGRAM_CACHE[key]


def _pack_w1(w: np.ndarray) -> np.ndarray:
    # [HIDDEN, GU] f32 -> [GB, P, KH, P] bf16; row h = 128*k + p, col = 128*b + j
    return np.ascontiguousarray(
        w.reshape(KH, P, GB, P).transpose(2, 1, 0, 3).astype(BF16)
    )


def _pack_w2(w: np.ndarray) -> np.ndarray:
    # [INTER, HIDDEN] f32 -> [MO, P, KI, P]; row f = 128*k + p, col = 128*m + j
    return np.ascontiguousarray(
        w.reshape(KI, P, MO, P).transpose(2, 1, 0, 3).astype(BF16)
    )


def _mixed_cover(counts, slot_sizes):
    """Exact-cover counts by pieces {size: N_CORES per size}. Returns
    per-core shard lists [(expert, row0, nrows), ...] ordered like
    slot_sizes, or None if no exact cover exists."""
    from collections import Counter

    sizes_desc = sorted(slot_sizes, reverse=True)
    avail = Counter(slot_sizes)
    for s in avail:
        avail[s] *= N_CORES
    per_expert: list = [None] * len(counts)

    def cover(rem, max_size):
        if rem == 0:
            return []
        for s in sorted(set(avail), reverse=True):
            if s > max_size or s > rem or avail[s] == 0:
                continue
            avail[s] -= 1
            sub = cover(rem - s, s)
            if sub is not None:
                return [s] + sub
            avail[s] += 1
        return None

    # Largest counts first so big pieces go where they must.
    order = sorted(range(len(counts)), key=lambda e: -counts[e])
    for e in order:
        pieces = cover(counts[e], max(sizes_desc))
        if pieces is None:
            return None
        per_expert[e] = pieces

    # Build shard pieces and deal them out per size class.
    by_size: dict = {s: [] for s in set(slot_sizes)}
    for e in range(len(counts)):
        r = 0
        for s in sorted(per_expert[e], reverse=True):
            by_size[s].append((e, r, s))
            r += s
    # Pad classes with empty shards (possible when sum(counts) is short).
    for s, lst in by_size.items():
        want = slot_sizes.count(s) * N_CORES
        while len(lst) < want:
            lst.append((0, 0, 0))
        if len(lst) != want:
            return None

    cores = []
    for r in range(N_CORES):
        shards = []
        used = {s: 0 for s in by_size}
        for s in slot_sizes:
            shards.append(by_size[s][r * slot_sizes.count(s) + used[s]])
            used[s] += 1
        cores.append(shards)
    return cores


def _uniform_cover(counts, slot):
    shards = []
    for e in range(len(counts)):
        r = 0
        while r < counts[e]:
            n = min(slot, counts[e] - r)
            shards.append((e, r, n))
            r += n
    n_slots = max(1, math.ceil(len(shards) / N_CORES))
    while len(shards) < N_CORES * n_slots:
        shards.append((0, 0, 0))
    return [shards[r * n_slots : (r + 1) * n_slots] for r in range(N_CORES)], n_slots


def _run(
    hidden_states: np.ndarray,
    merged_gate_up_proj: np.ndarray,
    merged_down_proj: np.ndarray,
    num_tokens_per_expert: np.ndarray,
    trace: bool = False,
):
    from concourse.bass_utils import run_bass_kernel_spmd

    counts = [int(c) for c in np.asarray(num_tokens_per_expert)]
    n_experts = len(counts)
    offs = np.concatenate([[0], np.cumsum(counts)]).astype(int)
    total = int(offs[-1])

    core_shards = _mixed_cover(counts, MIXED_SLOTS)
    if core_shards is not None:
        slot_sizes = MIXED_SLOTS
    else:
        core_shards, n_slots = _uniform_cover(counts, UNIFORM_SLOT)
        slot_sizes = (UNIFORM_SLOT,) * n_slots

    slot_off = np.concatenate([[0], np.cumsum(slot_sizes)]).astype(int)
    T = int(slot_off[-1])

    nc = _get_program(slot_sizes, NT)

    w1_packed = [_pack_w1(merged_gate_up_proj[e]) for e in range(n_experts)]
    w2_packed = [_pack_w2(merged_down_proj[e]) for e in range(n_experts)]
    x_bf16 = hidden_states.astype(BF16)

    in_maps = []
    for r in range(N_CORES):
        shards = core_shards[r]
        x_core = np.zeros((T, HIDDEN), dtype=BF16)
        for s, (e, r0, n) in enumerate(shards):
            if n:
                x_core[slot_off[s] : slot_off[s] + n] = x_bf16[
                    offs[e] + r0 : offs[e] + r0 + n
                ]
        # [T, HIDDEN] -> [P, KH, T] with column h = 128*k + p
        xT_core = np.ascontiguousarray(
            x_core.T.reshape(KH, P, T).transpose(1, 0, 2)
        )
        in_maps.append(
            {
                "xT": xT_core,
                "w1": np.stack([w1_packed[e] for (e, _, _) in shards]),
                "w2": np.stack([w2_packed[e] for (e, _, _) in shards]),
            }
        )

    res = run_bass_kernel_spmd(nc, in_maps, list(range(N_CORES)), trace=trace)

    out = np.empty((total, HIDDEN), dtype=np.float32)
    for r in range(N_CORES):
        # [P, MO, T] -> [T, HIDDEN] with column o = 128*m + p
        o_core = (
            res.results[r]["outT"].transpose(2, 1, 0).reshape(T, HIDDEN)
        ).astype(np.float32)
        for s, (e, r0, n) in enumerate(core_shards[r]):
            if n:
                out[offs[e] + r0 : offs[e] + r0 + n] = o_core[
                    slot_off[s] : slot_off[s] + n
                ]
    return out, res


def kernel(**inputs) -> np.ndarray:
    return _run(**inputs, trace=False)[0]


def run_traced(**inputs):
    return _run(**inputs, trace=True)


# revision 9
# speedup vs baseline: 1.0170x; 1.0170x over previous
"""Grouped MoE MLP (SwiGLU) kernel for Trainium2, 8 NeuronCores.

Strategy (expert-parallel, host-side routing):
  Tokens arrive pre-sorted by expert with per-expert counts.  The host
  partitions each expert's token block into pieces matching a fixed
  per-core slot structure (preferring the zero-padding mixed cover
  (1024, 512, 256, 256) = 2048 rows/core, falling back to a uniform
  768-row scheme), and gathers the matching expert weights per
  (core, slot).  Every core runs the identical program: for each slot,
  a dense SwiGLU MLP of that slot's tokens with that slot's expert
  weights.  No device-side routing or collectives are needed.

  Layouts are transposed on the host so both GEMMs contract over the
  SBUF partition dimension with no on-chip transposes:
    GEMM1: out1^T[f, t] = sum_h W1[h, f] * x[t, h]   (h on partitions)
    SwiGLU on feature-partitioned tiles
    GEMM2: out^T[o, t]  = sum_f W2[f, o] * h[t, f]   (f on partitions)

  Weights are packed on the host into per-128-column blocks
  (w1: [slot, 22, P, KH, 128], w2: [slot, 16, P, KI, 128]) and loaded
  on-chip as small 0.72MB/0.36MB tiles in deep rings.  GEMM1 runs
  mp-outer / chunk-inner so each weight block's last use comes early
  and the ring continuously prefetches across slot boundaries --
  keeping the PE array busy (and the HAM clock-gate warm) end to end.
  Output is stored bf16 to halve the output DMA traffic.
"""

import math
from contextlib import ExitStack

import ml_dtypes
import numpy as np

P = 128
HIDDEN = 2048
INTER = 1408
GU = 2 * INTER            # 2816 = gate+up columns
KH = HIDDEN // P          # 16 k-tiles for GEMM1
KI = INTER // P           # 11 k-tiles for GEMM2 / gate-up pair blocks
MO = HIDDEN // P          # 16 output feature blocks
GB = GU // P              # 22 gate+up column blocks
N_CORES = 8
NT = 512                  # max tokens per chunk (matmul moving free dim)
# zero-padding cover, 2048 rows/core.  Largest slot first: the small
# slots are locally DMA-bound (weight load ~51us vs ~56us compute), so
# the big compute-bound slot runs first and builds up DMA-stream slack
# that carries the small slots at the tail without starving the PE.
MIXED_SLOTS = (1024, 512, 256, 256)
UNIFORM_SLOT = 768                    # fallback slot size

BF16 = ml_dtypes.bfloat16

_PROGRAM_CACHE: dict = {}


def _chunks(slot_rows: int, nt: int):
    out = []
    r = 0
    while r < slot_rows:
        c = min(nt, slot_rows - r)
        out.append((r, c))
        r += c
    return out


def _build_program(slot_sizes: tuple, nt: int):
    import concourse.mybir as mybir
    import concourse.tile as tile
    from concourse import bacc

    n_slots = len(slot_sizes)
    T = sum(slot_sizes)
    slot_off = np.concatenate([[0], np.cumsum(slot_sizes)]).astype(int)
    bf16 = mybir.dt.bfloat16
    f32 = mybir.dt.float32

    nc = bacc.Bacc(None, target_bir_lowering=False, debug=False)
    xT = nc.dram_tensor("xT", [P, KH, T], bf16, kind="ExternalInput")
    w1 = nc.dram_tensor("w1", [n_slots, GB, P, KH, P], bf16, kind="ExternalInput")
    w2 = nc.dram_tensor("w2", [n_slots, MO, P, KI, P], bf16, kind="ExternalInput")
    outT = nc.dram_tensor("outT", [P, MO, T], bf16, kind="ExternalOutput")

    with tile.TileContext(nc) as tc, ExitStack() as ctx:
        w1_pool = ctx.enter_context(tc.tile_pool(name="w1p", bufs=6))
        w2_pool = ctx.enter_context(tc.tile_pool(name="w2p", bufs=6))
        x_pool = ctx.enter_context(tc.tile_pool(name="xp", bufs=4))
        h_pool = ctx.enter_context(tc.tile_pool(name="hp", bufs=2))
        g_pool = ctx.enter_context(tc.tile_pool(name="gp", bufs=3))
        o_pool = ctx.enter_context(tc.tile_pool(name="op", bufs=4))
        ps1 = ctx.enter_context(tc.tile_pool(name="ps1", bufs=2, space="PSUM"))
        ps2 = ctx.enter_context(tc.tile_pool(name="ps2", bufs=4, space="PSUM"))

        def emit_gemm2_group(w2m, ht, m, chunk_list, base):
            for c_off, c_n in chunk_list:
                po = ps2.tile([P, c_n], f32, tag="po")
                for k in range(KI):
                    nc.tensor.matmul(
                        po[:], w2m[:, k, :], ht[:, k, c_off : c_off + c_n],
                        start=(k == 0), stop=(k == KI - 1),
                    )
                om = o_pool.tile([P, c_n], bf16, tag="om")
                nc.vector.tensor_copy(om[:], po[:])
                nc.scalar.dma_start(
                    outT[:, m, base + c_off : base + c_off + c_n], om[:]
                )

        # GEMM2 groups held back from the previous slot: emitted right
        # after the next slot's GEMM1 so the PE has ready work to chew
        # on while the last SwiGLU silu+mul latency drains.
        deferred: list = []
        N_DEFER = 2

        for s in range(n_slots):
            sz = slot_sizes[s]
            chunk_list = _chunks(sz, nt)
            base = int(slot_off[s])

            # DMA ring split: w1 weight stream owns the Sync HWDGE FIFO;
            # x / w2 / out ride the Scalar HWDGE FIFO so they never queue
            # ahead of the weight prefetch.
            wg0 = w1_pool.tile([P, KH, P], bf16, tag="w1")
            nc.sync.dma_start(wg0[:], w1[s, 0, :, :, :])
            wu0 = w1_pool.tile([P, KH, P], bf16, tag="w1")
            nc.sync.dma_start(wu0[:], w1[s, KI, :, :, :])

            xts = []
            for c_off, c_n in chunk_list:
                xt = x_pool.tile([P, KH, c_n], bf16, tag="xt")
                nc.scalar.dma_start(xt[:], xT[:, :, base + c_off : base + c_off + c_n])
                xts.append(xt)

            ht = h_pool.tile([P, KI, sz], bf16, tag="ht")
            for mp in range(KI):
                if mp == 0:
                    wg, wu = wg0, wu0
                else:
                    wg = w1_pool.tile([P, KH, P], bf16, tag="w1")
                    nc.sync.dma_start(wg[:], w1[s, mp, :, :, :])
                    wu = w1_pool.tile([P, KH, P], bf16, tag="w1")
                    nc.sync.dma_start(wu[:], w1[s, KI + mp, :, :, :])
                for ci, (c_off, c_n) in enumerate(chunk_list):
                    xt = xts[ci]
                    pg = ps1.tile([P, c_n], f32, tag="pg")
                    pu = ps1.tile([P, c_n], f32, tag="pu")
                    for k in range(KH):
                        nc.tensor.matmul(
                            pg[:], wg[:, k, :], xt[:, k, :],
                            start=(k == 0), stop=(k == KH - 1),
                        )
                    for k in range(KH):
                        nc.tensor.matmul(
                            pu[:], wu[:, k, :], xt[:, k, :],
                            start=(k == 0), stop=(k == KH - 1),
                        )
                    gt = g_pool.tile([P, c_n], bf16, tag="gt")
                    nc.scalar.activation(
                        gt[:], pg[:], mybir.ActivationFunctionType.Silu
                    )
                    nc.vector.tensor_mul(
                        ht[:, mp, c_off : c_off + c_n], gt[:], pu[:]
                    )

            for w2m_d, ht_d, m_d, cl_d, base_d in deferred:
                emit_gemm2_group(w2m_d, ht_d, m_d, cl_d, base_d)
            deferred = []

            for m in range(MO):
                w2m = w2_pool.tile([P, KI, P], bf16, tag="w2")
                nc.scalar.dma_start(w2m[:], w2[s, m, :, :, :])
                if m >= MO - N_DEFER and s < n_slots - 1:
                    deferred.append((w2m, ht, m, chunk_list, base))
                else:
                    emit_gemm2_group(w2m, ht, m, chunk_list, base)

        for w2m_d, ht_d, m_d, cl_d, base_d in deferred:
            emit_gemm2_group(w2m_d, ht_d, m_d, cl_d, base_d)
    nc.compile()
    return nc


def _get_program(slot_sizes: tuple, nt: int):
    key = (tuple(slot_sizes), nt)
    if key not in _PROGRAM_CACHE:
        _PROGRAM_CACHE[key] = _build_program(tuple(slot_sizes), nt)
    return _PROGRAM_CACHE[key]


def _pack_w1(w: np.ndarray) -> np.ndarray:
    # [HIDDEN, GU] f32 -> [GB, P, KH, P] bf16; row h = 128*k + p, col = 128*b + j
    return np.ascontiguousarray(
        w.reshape(KH, P, GB, P).transpose(2, 1, 0, 3).astype(BF16)
    )


def _pack_w2(w: np.ndarray) -> np.ndarray:
    # [INTER, HIDDEN] f32 -> [MO, P, KI, P]; row f = 128*k + p, col = 128*m + j
    return np.ascontiguousarray(
        w.reshape(KI, P, MO, P).transpose(2, 1, 0, 3).astype(BF16)
    )


def _mixed_cover(counts, slot_sizes):
    """Exact-cover counts by pieces {size: N_CORES per size}. Returns
    per-core shard lists [(expert, row0, nrows), ...] ordered like
    slot_sizes, or None if no exact cover exists."""
    from collections import Counter

    sizes_desc = sorted(slot_sizes, reverse=True)
    avail = Counter(slot_sizes)
    for s in avail:
        avail[s] *= N_CORES
    per_expert: list = [None] * len(counts)

    def cover(rem, max_size):
        if rem == 0:
            return []
        for s in sorted(set(avail), reverse=True):
            if s > max_size or s > rem or avail[s] == 0:
                continue
            avail[s] -= 1
            sub = cover(rem - s, s)
            if sub is not None:
                return [s] + sub
            avail[s] += 1
        return None

    # Largest counts first so big pieces go where they must.
    order = sorted(range(len(counts)), key=lambda e: -counts[e])
    for e in order:
        pieces = cover(counts[e], max(sizes_desc))
        if pieces is None:
            return None
        per_expert[e] = pieces

    # Build shard pieces and deal them out per size class.
    by_size: dict = {s: [] for s in set(slot_sizes)}
    for e in range(len(counts)):
        r = 0
        for s in sorted(per_expert[e], reverse=True):
            by_size[s].append((e, r, s))
            r += s
    # Pad classes with empty shards (possible when sum(counts) is short).
    for s, lst in by_size.items():
        want = slot_sizes.count(s) * N_CORES
        while len(lst) < want:
            lst.append((0, 0, 0))
        if len(lst) != want:
            return None

    cores = []
    for r in range(N_CORES):
        shards = []
        used = {s: 0 for s in by_size}
        for s in slot_sizes:
            shards.append(by_size[s][r * slot_sizes.count(s) + used[s]])
            used[s] += 1
        cores.append(shards)
    return cores


def _uniform_cover(counts, slot):
    shards = []
    for e in range(len(counts)):
        r = 0
        while r < counts[e]:
            n = min(slot, counts[e] - r)
            shards.append((e, r, n))
            r += n
    n_slots = max(1, math.ceil(len(shards) / N_CORES))
    while len(shards) < N_CORES * n_slots:
        shards.append((0, 0, 0))
    return [shards[r * n_slots : (r + 1) * n_slots] for r in range(N_CORES)], n_slots


def _run(
    hidden_states: np.ndarray,
    merged_gate_up_proj: np.ndarray,
    merged_down_proj: np.ndarray,
    num_tokens_per_expert: np.ndarray,
    trace: bool = False,
):
    from concourse.bass_utils import run_bass_kernel_spmd

    counts = [int(c) for c in np.asarray(num_tokens_per_expert)]
    n_experts = len(counts)
    offs = np.concatenate([[0], np.cumsum(counts)]).astype(int)
    total = int(offs[-1])

    core_shards = _mixed_cover(counts, MIXED_SLOTS)
    if core_shards is not None:
        slot_sizes = MIXED_SLOTS
    else:
        core_shards, n_slots = _uniform_cover(counts, UNIFORM_SLOT)
        slot_sizes = (UNIFORM_SLOT,) * n_slots

    slot_off = np.concatenate([[0], np.cumsum(slot_sizes)]).astype(int)
    T = int(slot_off[-1])

    nc = _get_program(slot_sizes, NT)

    w1_packed = [_pack_w1(merged_gate_up_proj[e]) for e in range(n_experts)]
    w2_packed = [_pack_w2(merged_down_proj[e]) for e in range(n_experts)]
    x_bf16 = hidden_states.astype(BF16)

    in_maps = []
    for r in range(N_CORES):
        shards = core_shards[r]
        x_core = np.zeros((T, HIDDEN), dtype=BF16)
        for s, (e, r0, n) in enumerate(shards):
            if n:
                x_core[slot_off[s] : slot_off[s] + n] = x_bf16[
                    offs[e] + r0 : offs[e] + r0 + n
                ]
        # [T, HIDDEN] -> [P, KH, T] with column h = 128*k + p
        xT_core = np.ascontiguousarray(
            x_core.T.reshape(KH, P, T).transpose(1, 0, 2)
        )
        in_maps.append(
            {
                "xT": xT_core,
                "w1": np.stack([w1_packed[e] for (e, _, _) in shards]),
                "w2": np.stack([w2_packed[e] for (e, _, _) in shards]),
            }
        )

    res = run_bass_kernel_spmd(nc, in_maps, list(range(N_CORES)), trace=trace)

    out = np.empty((total, HIDDEN), dtype=np.float32)
    for r in range(N_CORES):
        # [P, MO, T] -> [T, HIDDEN] with column o = 128*m + p
        o_core = (
            res.results[r]["outT"].transpose(2, 1, 0).reshape(T, HIDDEN)
        ).astype(np.float32)
        for s, (e, r0, n) in enumerate(core_shards[r]):
            if n:
                out[offs[e] + r0 : offs[e] + r0 + n] = o_core[
                    slot_off[s] : slot_off[s] + n
                ]
    return out, res


def kernel(**inputs) -> np.ndarray:
    return _run(**inputs, trace=False)[0]


def run_traced(**inputs):
    return _run(**inputs, trace=True)


# revision 11
# speedup vs baseline: 1.0175x; 1.0005x over previous
"""Grouped MoE MLP (SwiGLU) kernel for Trainium2, 8 NeuronCores.

Strategy (expert-parallel, host-side routing):
  Tokens arrive pre-sorted by expert with per-expert counts.  The host
  partitions each expert's token block into pieces matching a fixed
  per-core slot structure (preferring the zero-padding mixed cover
  (1024, 512, 256, 256) = 2048 rows/core, falling back to a uniform
  768-row scheme), and gathers the matching expert weights per
  (core, slot).  Every core runs the identical program: for each slot,
  a dense SwiGLU MLP of that slot's tokens with that slot's expert
  weights.  No device-side routing or collectives are needed.

  Layouts are transposed on the host so both GEMMs contract over the
  SBUF partition dimension with no on-chip transposes:
    GEMM1: out1^T[f, t] = sum_h W1[h, f] * x[t, h]   (h on partitions)
    SwiGLU on feature-partitioned tiles
    GEMM2: out^T[o, t]  = sum_f W2[f, o] * h[t, f]   (f on partitions)

  Weights are packed on the host into per-128-column blocks
  (w1: [slot, 22, P, KH, 128], w2: [slot, 16, P, KI, 128]) and loaded
  on-chip as small 0.72MB/0.36MB tiles in deep rings.  GEMM1 runs
  mp-outer / chunk-inner so each weight block's last use comes early
  and the ring continuously prefetches across slot boundaries --
  keeping the PE array busy (and the HAM clock-gate warm) end to end.
  Output is stored bf16 to halve the output DMA traffic.
"""

import math
from contextlib import ExitStack

import ml_dtypes
import numpy as np

P = 128
HIDDEN = 2048
INTER = 1408
GU = 2 * INTER            # 2816 = gate+up columns
KH = HIDDEN // P          # 16 k-tiles for GEMM1
KI = INTER // P           # 11 k-tiles for GEMM2 / gate-up pair blocks
MO = HIDDEN // P          # 16 output feature blocks
GB = GU // P              # 22 gate+up column blocks
N_CORES = 8
NT = 512                  # max tokens per chunk (matmul moving free dim)
# zero-padding cover, 2048 rows/core.  Largest slot first: the small
# slots are locally DMA-bound (weight load ~51us vs ~56us compute), so
# the big compute-bound slot runs first and builds up DMA-stream slack
# that carries the small slots at the tail without starving the PE.
MIXED_SLOTS = (1024, 512, 256, 256)
UNIFORM_SLOT = 768                    # fallback slot size

BF16 = ml_dtypes.bfloat16

_PROGRAM_CACHE: dict = {}


def _chunks(slot_rows: int, nt: int):
    out = []
    r = 0
    while r < slot_rows:
        c = min(nt, slot_rows - r)
        out.append((r, c))
        r += c
    return out


def _build_program(slot_sizes: tuple, nt: int):
    import concourse.mybir as mybir
    import concourse.tile as tile
    from concourse import bacc

    n_slots = len(slot_sizes)
    T = sum(slot_sizes)
    slot_off = np.concatenate([[0], np.cumsum(slot_sizes)]).astype(int)
    bf16 = mybir.dt.bfloat16
    f32 = mybir.dt.float32

    nc = bacc.Bacc(None, target_bir_lowering=False, debug=False)
    xT = nc.dram_tensor("xT", [P, KH, T], bf16, kind="ExternalInput")
    w1 = nc.dram_tensor("w1", [n_slots, GB, P, KH, P], bf16, kind="ExternalInput")
    w2 = nc.dram_tensor("w2", [n_slots, MO, P, KI, P], bf16, kind="ExternalInput")
    outT = nc.dram_tensor("outT", [P, MO, T], bf16, kind="ExternalOutput")

    with tile.TileContext(nc) as tc, ExitStack() as ctx:
        w1_pool = ctx.enter_context(tc.tile_pool(name="w1p", bufs=24))
        w2_pool = ctx.enter_context(tc.tile_pool(name="w2p", bufs=6))
        x_pool = ctx.enter_context(tc.tile_pool(name="xp", bufs=2))
        h_pool = ctx.enter_context(tc.tile_pool(name="hp", bufs=2))
        g_pool = ctx.enter_context(tc.tile_pool(name="gp", bufs=3))
        o_pool = ctx.enter_context(tc.tile_pool(name="op", bufs=4))
        ps1 = ctx.enter_context(tc.tile_pool(name="ps1", bufs=2, space="PSUM"))
        ps2 = ctx.enter_context(tc.tile_pool(name="ps2", bufs=4, space="PSUM"))

        def emit_gemm2_group(w2m, ht, m, chunk_list, base):
            for c_off, c_n in chunk_list:
                po = ps2.tile([P, c_n], f32, tag="po")
                for k in range(KI):
                    nc.tensor.matmul(
                        po[:], w2m[:, k, :], ht[:, k, c_off : c_off + c_n],
                        start=(k == 0), stop=(k == KI - 1),
                    )
                om = o_pool.tile([P, c_n], bf16, tag="om")
                nc.vector.tensor_copy(om[:], po[:])
                nc.scalar.dma_start(
                    outT[:, m, base + c_off : base + c_off + c_n], om[:]
                )

        # GEMM2 groups held back from the previous slot: emitted right
        # after the next slot's GEMM1 so the PE has ready work to chew
        # on while the last SwiGLU silu+mul latency drains.
        deferred: list = []
        N_DEFER = 2

        for s in range(n_slots):
            sz = slot_sizes[s]
            chunk_list = _chunks(sz, nt)
            base = int(slot_off[s])

            # DMA ring split: the w1 weight stream owns the Sync HWDGE
            # FIFO, x rides the GpSimd SWDGE ring, and w2 / out ride the
            # Scalar HWDGE FIFO -- so no stream ever queues ahead of the
            # weight prefetch and x is never stuck behind w2/out.
            # GEMM1 is chunk-outer / mp-inner: the first chunk only
            # needs one x chunk + one weight pair before the PE starts,
            # and the slot's 22 w1 blocks stay resident (bufs=24 ring)
            # for the later chunks while the next slot's blocks stream
            # in behind them.
            xts = []
            for c_off, c_n in chunk_list:
                xt = x_pool.tile([P, KH, c_n], bf16, tag="xt")
                nc.gpsimd.dma_start(xt[:], xT[:, :, base + c_off : base + c_off + c_n])
                xts.append(xt)

            w1ts = []
            for mp in range(KI):
                wg = w1_pool.tile([P, KH, P], bf16, tag="w1")
                nc.sync.dma_start(wg[:], w1[s, mp, :, :, :])
                wu = w1_pool.tile([P, KH, P], bf16, tag="w1")
                nc.sync.dma_start(wu[:], w1[s, KI + mp, :, :, :])
                w1ts.append((wg, wu))

            ht = h_pool.tile([P, KI, sz], bf16, tag="ht")
            for ci, (c_off, c_n) in enumerate(chunk_list):
                xt = xts[ci]
                for mp in range(KI):
                    wg, wu = w1ts[mp]
                    pg = ps1.tile([P, c_n], f32, tag="pg")
                    pu = ps1.tile([P, c_n], f32, tag="pu")
                    for k in range(KH):
                        nc.tensor.matmul(
                            pg[:], wg[:, k, :], xt[:, k, :],
                            start=(k == 0), stop=(k == KH - 1),
                        )
                    for k in range(KH):
                        nc.tensor.matmul(
                            pu[:], wu[:, k, :], xt[:, k, :],
                            start=(k == 0), stop=(k == KH - 1),
                        )
                    gt = g_pool.tile([P, c_n], bf16, tag="gt")
                    nc.scalar.activation(
                        gt[:], pg[:], mybir.ActivationFunctionType.Silu
                    )
                    nc.vector.tensor_mul(
                        ht[:, mp, c_off : c_off + c_n], gt[:], pu[:]
                    )

            for w2m_d, ht_d, m_d, cl_d, base_d in deferred:
                emit_gemm2_group(w2m_d, ht_d, m_d, cl_d, base_d)
            deferred = []

            for m in range(MO):
                w2m = w2_pool.tile([P, KI, P], bf16, tag="w2")
                nc.scalar.dma_start(w2m[:], w2[s, m, :, :, :])
                if m >= MO - N_DEFER and s < n_slots - 1:
                    deferred.append((w2m, ht, m, chunk_list, base))
                else:
                    emit_gemm2_group(w2m, ht, m, chunk_list, base)

        for w2m_d, ht_d, m_d, cl_d, base_d in deferred:
            emit_gemm2_group(w2m_d, ht_d, m_d, cl_d, base_d)
    nc.compile()
    return nc


def _get_program(slot_sizes: tuple, nt: int):
    key = (tuple(slot_sizes), nt)
    if key not in _PROGRAM_CACHE:
        _PROGRAM_CACHE[key] = _build_program(tuple(slot_sizes), nt)
    return _PROGRAM_CACHE[key]


def _pack_w1(w: np.ndarray) -> np.ndarray:
    # [HIDDEN, GU] f32 -> [GB, P, KH, P] bf16; row h = 128*k + p, col = 128*b + j
    return np.ascontiguousarray(
        w.reshape(KH, P, GB, P).transpose(2, 1, 0, 3).astype(BF16)
    )


def _pack_w2(w: np.ndarray) -> np.ndarray:
    # [INTER, HIDDEN] f32 -> [MO, P, KI, P]; row f = 128*k + p, col = 128*m + j
    return np.ascontiguousarray(
        w.reshape(KI, P, MO, P).transpose(2, 1, 0, 3).astype(BF16)
    )


def _mixed_cover(counts, slot_sizes):
    """Exact-cover counts by pieces {size: N_CORES per size}. Returns
    per-core shard lists [(expert, row0, nrows), ...] ordered like
    slot_sizes, or None if no exact cover exists."""
    from collections import Counter

    sizes_desc = sorted(slot_sizes, reverse=True)
    avail = Counter(slot_sizes)
    for s in avail:
        avail[s] *= N_CORES
    per_expert: list = [None] * len(counts)

    def cover(rem, max_size):
        if rem == 0:
            return []
        for s in sorted(set(avail), reverse=True):
            if s > max_size or s > rem or avail[s] == 0:
                continue
            avail[s] -= 1
            sub = cover(rem - s, s)
            if sub is not None:
                return [s] + sub
            avail[s] += 1
        return None

    # Largest counts first so big pieces go where they must.
    order = sorted(range(len(counts)), key=lambda e: -counts[e])
    for e in order:
        pieces = cover(counts[e], max(sizes_desc))
        if pieces is None:
            return None
        per_expert[e] = pieces

    # Build shard pieces and deal them out per size class.
    by_size: dict = {s: [] for s in set(slot_sizes)}
    for e in range(len(counts)):
        r = 0
        for s in sorted(per_expert[e], reverse=True):
            by_size[s].append((e, r, s))
            r += s
    # Pad classes with empty shards (possible when sum(counts) is short).
    for s, lst in by_size.items():
        want = slot_sizes.count(s) * N_CORES
        while len(lst) < want:
            lst.append((0, 0, 0))
        if len(lst) != want:
            return None

    cores = []
    for r in range(N_CORES):
        shards = []
        used = {s: 0 for s in by_size}
        for s in slot_sizes:
            shards.append(by_size[s][r * slot_sizes.count(s) + used[s]])
            used[s] += 1
        cores.append(shards)
    return cores


def _uniform_cover(counts, slot):
    shards = []
    for e in range(len(counts)):
        r = 0
        while r < counts[e]:
            n = min(slot, counts[e] - r)
            shards.append((e, r, n))
            r += n
    n_slots = max(1, math.ceil(len(shards) / N_CORES))
    while len(shards) < N_CORES * n_slots:
        shards.append((0, 0, 0))
    return [shards[r * n_slots : (r + 1) * n_slots] for r in range(N_CORES)], n_slots


def _run(
    hidden_states: np.ndarray,
    merged_gate_up_proj: np.ndarray,
    merged_down_proj: np.ndarray,
    num_tokens_per_expert: np.ndarray,
    trace: bool = False,
):
    from concourse.bass_utils import run_bass_kernel_spmd

    counts = [int(c) for c in np.asarray(num_tokens_per_expert)]
    n_experts = len(counts)
    offs = np.concatenate([[0], np.cumsum(counts)]).astype(int)
    total = int(offs[-1])

    core_shards = _mixed_cover(counts, MIXED_SLOTS)
    if core_shards is not None:
        slot_sizes = MIXED_SLOTS
    else:
        core_shards, n_slots = _uniform_cover(counts, UNIFORM_SLOT)
        slot_sizes = (UNIFORM_SLOT,) * n_slots

    slot_off = np.concatenate([[0], np.cumsum(slot_sizes)]).astype(int)
    T = int(slot_off[-1])

    nc = _get_program(slot_sizes, NT)

    w1_packed = [_pack_w1(merged_gate_up_proj[e]) for e in range(n_experts)]
    w2_packed = [_pack_w2(merged_down_proj[e]) for e in range(n_experts)]
    x_bf16 = hidden_states.astype(BF16)

    in_maps = []
    for r in range(N_CORES):
        shards = core_shards[r]
        x_core = np.zeros((T, HIDDEN), dtype=BF16)
        for s, (e, r0, n) in enumerate(shards):
            if n:
                x_core[slot_off[s] : slot_off[s] + n] = x_bf16[
                    offs[e] + r0 : offs[e] + r0 + n
                ]
        # [T, HIDDEN] -> [P, KH, T] with column h = 128*k + p
        xT_core = np.ascontiguousarray(
            x_core.T.reshape(KH, P, T).transpose(1, 0, 2)
        )
        in_maps.append(
            {
                "xT": xT_core,
                "w1": np.stack([w1_packed[e] for (e, _, _) in shards]),
                "w2": np.stack([w2_packed[e] for (e, _, _) in shards]),
            }
        )

    res = run_bass_kernel_spmd(nc, in_maps, list(range(N_CORES)), trace=trace)

    out = np.empty((total, HIDDEN), dtype=np.float32)
    for r in range(N_CORES):
        # [P, MO, T] -> [T, HIDDEN] with column o = 128*m + p
        o_core = (
            res.results[r]["outT"].transpose(2, 1, 0).reshape(T, HIDDEN)
        ).astype(np.float32)
        for s, (e, r0, n) in enumerate(core_shards[r]):
            if n:
                out[offs[e] + r0 : offs[e] + r0 + n] = o_core[
                    slot_off[s] : slot_off[s] + n
                ]
    return out, res


def kernel(**inputs) -> np.ndarray:
    return _run(**inputs, trace=False)[0]


def run_traced(**inputs):
    return _run(**inputs, trace=True)


# revision 13
# speedup vs baseline: 1.0610x; 1.0427x over previous
"""Grouped MoE MLP (SwiGLU) kernel for Trainium2, 8 NeuronCores.

Strategy (expert-parallel, host-side routing):
  Tokens arrive pre-sorted by expert with per-expert counts.  The host
  partitions each expert's token block into pieces matching a fixed
  per-core slot structure (preferring the zero-padding mixed cover
  (1024, 512, 256, 256) = 2048 rows/core, falling back to a uniform
  768-row scheme), and gathers the matching expert weights per
  (core, slot).  Every core runs the identical program: for each slot,
  a dense SwiGLU MLP of that slot's tokens with that slot's expert
  weights.  No device-side routing or collectives are needed.

  Layouts are transposed on the host so both GEMMs contract over the
  SBUF partition dimension with no on-chip transposes:
    GEMM1: out1^T[f, t] = sum_h W1[h, f] * x[t, h]   (h on partitions)
    SwiGLU on feature-partitioned tiles
    GEMM2: out^T[o, t]  = sum_f W2[f, o] * h[t, f]   (f on partitions)

  Weights are packed on the host into per-128-column blocks
  (w1: [slot, 22, P, KH, 128], w2: [slot, 16, P, KI, 128]) and loaded
  on-chip as small 0.72MB/0.36MB tiles in deep rings.  GEMM1 runs
  mp-outer / chunk-inner so each weight block's last use comes early
  and the ring continuously prefetches across slot boundaries --
  keeping the PE array busy (and the HAM clock-gate warm) end to end.
  Output is stored bf16 to halve the output DMA traffic.
"""

import math
from contextlib import ExitStack

import ml_dtypes
import numpy as np

P = 128
HIDDEN = 2048
INTER = 1408
GU = 2 * INTER            # 2816 = gate+up columns
KH = HIDDEN // P          # 16 k-tiles for GEMM1
KI = INTER // P           # 11 k-tiles for GEMM2 / gate-up pair blocks
MO = HIDDEN // P          # 16 output feature blocks
GB = GU // P              # 22 gate+up column blocks
N_CORES = 8
NT = 512                  # max tokens per chunk (matmul moving free dim)
# zero-padding cover, 2048 rows/core.  Largest slot first: the small
# slots are locally DMA-bound (weight load ~51us vs ~56us compute), so
# the big compute-bound slot runs first and builds up DMA-stream slack
# that carries the small slots at the tail without starving the PE.
MIXED_SLOTS = (1024, 512, 256, 256)
UNIFORM_SLOT = 768                    # fallback slot size

BF16 = ml_dtypes.bfloat16

_PROGRAM_CACHE: dict = {}


def _chunks(slot_rows: int, nt: int):
    out = []
    r = 0
    while r < slot_rows:
        c = min(nt, slot_rows - r)
        out.append((r, c))
        r += c
    return out


def _build_program(slot_sizes: tuple, nt: int):
    import concourse.mybir as mybir
    import concourse.tile as tile
    from concourse import bacc

    n_slots = len(slot_sizes)
    T = sum(slot_sizes)
    slot_off = np.concatenate([[0], np.cumsum(slot_sizes)]).astype(int)
    bf16 = mybir.dt.bfloat16
    f32 = mybir.dt.float32

    nc = bacc.Bacc(None, target_bir_lowering=False, debug=False)
    xT = nc.dram_tensor("xT", [P, KH, T], bf16, kind="ExternalInput")
    w1 = nc.dram_tensor("w1", [n_slots, GB, P, KH, P], bf16, kind="ExternalInput")
    w2 = nc.dram_tensor("w2", [n_slots, MO, P, KI, P], bf16, kind="ExternalInput")
    outT = nc.dram_tensor("outT", [P, MO, T], bf16, kind="ExternalOutput")

    with tile.TileContext(nc) as tc, ExitStack() as ctx:
        w1_pool = ctx.enter_context(tc.tile_pool(name="w1p", bufs=24))
        w2_pool = ctx.enter_context(tc.tile_pool(name="w2p", bufs=6))
        x_pool = ctx.enter_context(tc.tile_pool(name="xp", bufs=2))
        h_pool = ctx.enter_context(tc.tile_pool(name="hp", bufs=2))
        g_pool = ctx.enter_context(tc.tile_pool(name="gp", bufs=3))
        o_pool = ctx.enter_context(tc.tile_pool(name="op", bufs=4))
        ps1 = ctx.enter_context(tc.tile_pool(name="ps1", bufs=2, space="PSUM"))
        ps2 = ctx.enter_context(tc.tile_pool(name="ps2", bufs=4, space="PSUM"))

        def emit_gemm2_group(w2m, ht, m, chunk_list, base):
            for c_off, c_n in chunk_list:
                po = ps2.tile([P, c_n], f32, tag="po")
                for k in range(KI):
                    nc.tensor.matmul(
                        po[:], w2m[:, k, :], ht[:, k, c_off : c_off + c_n],
                        start=(k == 0), stop=(k == KI - 1),
                    )
                om = o_pool.tile([P, c_n], bf16, tag="om")
                nc.vector.tensor_copy(om[:], po[:])
                nc.scalar.dma_start(
                    outT[:, m, base + c_off : base + c_off + c_n], om[:]
                )

        # GEMM2 groups held back from the previous slot: emitted right
        # after the next slot's GEMM1 so the PE has ready work to chew
        # on while the last SwiGLU silu+mul latency drains.
        deferred: list = []
        N_DEFER = 2

        for s in range(n_slots):
            sz = slot_sizes[s]
            chunk_list = _chunks(sz, nt)
            base = int(slot_off[s])

            # All input DMAs ride the single Sync HWDGE FIFO in exact
            # consumption order -- the FIFO *is* the priority schedule,
            # so the HBM pipe always feeds the next thing the PE needs
            # (scheduler-reordered multi-ring splits steal bandwidth
            # from the critical stream).  Only output DMAs (naturally
            # gated on compute) ride the Scalar ring.
            # GEMM1 is chunk-outer / mp-inner: the first chunk needs
            # just one x chunk + one weight pair before the PE starts,
            # and the slot's 22 w1 blocks stay resident (bufs=24 ring)
            # for later chunks while the next slot's stream queues up
            # behind them.
            xts = []
            w1ts = []
            for mp in range(KI):
                wg = w1_pool.tile([P, KH, P], bf16, tag="w1")
                nc.sync.dma_start(wg[:], w1[s, mp, :, :, :])
                wu = w1_pool.tile([P, KH, P], bf16, tag="w1")
                nc.sync.dma_start(wu[:], w1[s, KI + mp, :, :, :])
                w1ts.append((wg, wu))
                # x chunks just-in-time: c0 right after the first weight
                # pair, later chunks a few pairs in.
                ci = len(xts)
                if ci < len(chunk_list) and mp >= 3 * ci:
                    c_off, c_n = chunk_list[ci]
                    xt = x_pool.tile([P, KH, c_n], bf16, tag="xt")
                    nc.sync.dma_start(
                        xt[:], xT[:, :, base + c_off : base + c_off + c_n]
                    )
                    xts.append(xt)
            for ci in range(len(xts), len(chunk_list)):
                c_off, c_n = chunk_list[ci]
                xt = x_pool.tile([P, KH, c_n], bf16, tag="xt")
                nc.sync.dma_start(xt[:], xT[:, :, base + c_off : base + c_off + c_n])
                xts.append(xt)

            ht = h_pool.tile([P, KI, sz], bf16, tag="ht")
            for ci, (c_off, c_n) in enumerate(chunk_list):
                xt = xts[ci]
                for mp in range(KI):
                    wg, wu = w1ts[mp]
                    pg = ps1.tile([P, c_n], f32, tag="pg")
                    pu = ps1.tile([P, c_n], f32, tag="pu")
                    for k in range(KH):
                        nc.tensor.matmul(
                            pg[:], wg[:, k, :], xt[:, k, :],
                            start=(k == 0), stop=(k == KH - 1),
                        )
                    for k in range(KH):
                        nc.tensor.matmul(
                            pu[:], wu[:, k, :], xt[:, k, :],
                            start=(k == 0), stop=(k == KH - 1),
                        )
                    gt = g_pool.tile([P, c_n], bf16, tag="gt")
                    nc.scalar.activation(
                        gt[:], pg[:], mybir.ActivationFunctionType.Silu
                    )
                    nc.vector.tensor_mul(
                        ht[:, mp, c_off : c_off + c_n], gt[:], pu[:]
                    )

            for w2m_d, ht_d, m_d, cl_d, base_d in deferred:
                emit_gemm2_group(w2m_d, ht_d, m_d, cl_d, base_d)
            deferred = []

            for m in range(MO):
                w2m = w2_pool.tile([P, KI, P], bf16, tag="w2")
                nc.sync.dma_start(w2m[:], w2[s, m, :, :, :])
                if m >= MO - N_DEFER and s < n_slots - 1:
                    deferred.append((w2m, ht, m, chunk_list, base))
                else:
                    emit_gemm2_group(w2m, ht, m, chunk_list, base)

        for w2m_d, ht_d, m_d, cl_d, base_d in deferred:
            emit_gemm2_group(w2m_d, ht_d, m_d, cl_d, base_d)
    nc.compile()
    return nc


def _get_program(slot_sizes: tuple, nt: int):
    key = (tuple(slot_sizes), nt)
    if key not in _PROGRAM_CACHE:
        _PROGRAM_CACHE[key] = _build_program(tuple(slot_sizes), nt)
    return _PROGRAM_CACHE[key]


def _pack_w1(w: np.ndarray) -> np.ndarray:
    # [HIDDEN, GU] f32 -> [GB, P, KH, P] bf16; row h = 128*k + p, col = 128*b + j
    return np.ascontiguousarray(
        w.reshape(KH, P, GB, P).transpose(2, 1, 0, 3).astype(BF16)
    )


def _pack_w2(w: np.ndarray) -> np.ndarray:
    # [INTER, HIDDEN] f32 -> [MO, P, KI, P]; row f = 128*k + p, col = 128*m + j
    return np.ascontiguousarray(
        w.reshape(KI, P, MO, P).transpose(2, 1, 0, 3).astype(BF16)
    )


def _mixed_cover(counts, slot_sizes):
    """Exact-cover counts by pieces {size: N_CORES per size}. Returns
    per-core shard lists [(expert, row0, nrows), ...] ordered like
    slot_sizes, or None if no exact cover exists."""
    from collections import Counter

    sizes_desc = sorted(slot_sizes, reverse=True)
    avail = Counter(slot_sizes)
    for s in avail:
        avail[s] *= N_CORES
    per_expert: list = [None] * len(counts)

    def cover(rem, max_size):
        if rem == 0:
            return []
        for s in sorted(set(avail), reverse=True):
            if s > max_size or s > rem or avail[s] == 0:
                continue
            avail[s] -= 1
            sub = cover(rem - s, s)
            if sub is not None:
                return [s] + sub
            avail[s] += 1
        return None

    # Largest counts first so big pieces go where they must.
    order = sorted(range(len(counts)), key=lambda e: -counts[e])
    for e in order:
        pieces = cover(counts[e], max(sizes_desc))
        if pieces is None:
            return None
        per_expert[e] = pieces

    # Build shard pieces and deal them out per size class.
    by_size: dict = {s: [] for s in set(slot_sizes)}
    for e in range(len(counts)):
        r = 0
        for s in sorted(per_expert[e], reverse=True):
            by_size[s].append((e, r, s))
            r += s
    # Pad classes with empty shards (possible when sum(counts) is short).
    for s, lst in by_size.items():
        want = slot_sizes.count(s) * N_CORES
        while len(lst) < want:
            lst.append((0, 0, 0))
        if len(lst) != want:
            return None

    cores = []
    for r in range(N_CORES):
        shards = []
        used = {s: 0 for s in by_size}
        for s in slot_sizes:
            shards.append(by_size[s][r * slot_sizes.count(s) + used[s]])
            used[s] += 1
        cores.append(shards)
    return cores


def _uniform_cover(counts, slot):
    shards = []
    for e in range(len(counts)):
        r = 0
        while r < counts[e]:
            n = min(slot, counts[e] - r)
            shards.append((e, r, n))
            r += n
    n_slots = max(1, math.ceil(len(shards) / N_CORES))
    while len(shards) < N_CORES * n_slots:
        shards.append((0, 0, 0))
    return [shards[r * n_slots : (r + 1) * n_slots] for r in range(N_CORES)], n_slots


def _run(
    hidden_states: np.ndarray,
    merged_gate_up_proj: np.ndarray,
    merged_down_proj: np.ndarray,
    num_tokens_per_expert: np.ndarray,
    trace: bool = False,
):
    from concourse.bass_utils import run_bass_kernel_spmd

    counts = [int(c) for c in np.asarray(num_tokens_per_expert)]
    n_experts = len(counts)
    offs = np.concatenate([[0], np.cumsum(counts)]).astype(int)
    total = int(offs[-1])

    core_shards = _mixed_cover(counts, MIXED_SLOTS)
    if core_shards is not None:
        slot_sizes = MIXED_SLOTS
    else:
        core_shards, n_slots = _uniform_cover(counts, UNIFORM_SLOT)
        slot_sizes = (UNIFORM_SLOT,) * n_slots

    slot_off = np.concatenate([[0], np.cumsum(slot_sizes)]).astype(int)
    T = int(slot_off[-1])

    nc = _get_program(slot_sizes, NT)

    w1_packed = [_pack_w1(merged_gate_up_proj[e]) for e in range(n_experts)]
    w2_packed = [_pack_w2(merged_down_proj[e]) for e in range(n_experts)]
    x_bf16 = hidden_states.astype(BF16)

    in_maps = []
    for r in range(N_CORES):
        shards = core_shards[r]
        x_core = np.zeros((T, HIDDEN), dtype=BF16)
        for s, (e, r0, n) in enumerate(shards):
            if n:
                x_core[slot_off[s] : slot_off[s] + n] = x_bf16[
                    offs[e] + r0 : offs[e] + r0 + n
                ]
        # [T, HIDDEN] -> [P, KH, T] with column h = 128*k + p
        xT_core = np.ascontiguousarray(
            x_core.T.reshape(KH, P, T).transpose(1, 0, 2)
        )
        in_maps.append(
            {
                "xT": xT_core,
                "w1": np.stack([w1_packed[e] for (e, _, _) in shards]),
                "w2": np.stack([w2_packed[e] for (e, _, _) in shards]),
            }
        )

    res = run_bass_kernel_spmd(nc, in_maps, list(range(N_CORES)), trace=trace)

    out = np.empty((total, HIDDEN), dtype=np.float32)
    for r in range(N_CORES):
        # [P, MO, T] -> [T, HIDDEN] with column o = 128*m + p
        o_core = (
            res.results[r]["outT"].transpose(2, 1, 0).reshape(T, HIDDEN)
        ).astype(np.float32)
        for s, (e, r0, n) in enumerate(core_shards[r]):
            if n:
                out[offs[e] + r0 : offs[e] + r0 + n] = o_core[
                    slot_off[s] : slot_off[s] + n
                ]
    return out, res


def kernel(**inputs) -> np.ndarray:
    return _run(**inputs, trace=False)[0]


def run_traced(**inputs):
    return _run(**inputs, trace=True)
